# revision 1
# baseline (speedup 1.0000x reference)
"""Trainium2 Bass kernel for the MultiHeadAttention problem.

Math (per head h):
  scores = (X Wq_h) (X Wk_h)^T * scale = X (scale * Wq_h Wk_h^T) X^T
so we precompute M_h = (scale*Wq_h) Wk_h^T once per head (batch independent),
then per batch:  TT = M_h^T X^T,  scoresT = X^T^T ... computed directly in the
transposed [m, n] orientation so that softmax's reduction axis (m) lands on
partitions and A can feed the A@V matmul without any transposes:
  scoresT[m, n] = sum_d' X[m, d'] TT[d', n]     (lhsT = X^T chunks, rhs = TT)
  A' = exp(scoresT - colmax)  -> HhT[v, n] = sum_m V[m, v] A'[m, n] / colsum
Output projection: Y_partial = concatT^T @ Wo_local, with the post-hoc triu
mask filled with LARGE_NEG/8 on every core so the host-side shard-sum yields
exactly LARGE_NEG at masked positions.

Sharding: 16 heads / 8 cores = 2 heads per core, every core handles all 4
batches; host sums the 8 partial outputs (the only cross-core reduction).

Precision: the score path (M, TT, scoresT) uses bf16 hi/lo split operands with
3 matmul passes (hi*hi + hi*lo + lo*hi) accumulated in fp32 PSUM, which keeps
score error at the fp32 level (verified: 0 argmax flips over all 65536 softmax
rows vs the fp64 ground truth). The linear paths (V, A@V, out-proj) use 2-3
bf16 passes, errors stay ~1e-3 relative, far inside tolerance.
"""

import os
import sys

import numpy as np
import ml_dtypes

for _p in ("/opt/trn_rl_repo",):
    if os.path.isdir(_p) and _p not in sys.path:
        sys.path.insert(0, _p)

import concourse.bass as bass
import concourse.tile as tile
from concourse import bacc, bass_isa, mybir

BF = mybir.dt.bfloat16
F32 = mybir.dt.float32
bf16 = ml_dtypes.bfloat16

# Problem constants (hardcoded per contract)
B, N, D, DV, H = 4, 1024, 1024, 64, 16
NCORES = 8
HLOC = H // NCORES  # heads per core
P = 128
FREE = 512  # PSUM free-dim limit for fp32 outputs
LARGE_NEG = -1.0e9


def _fsplits(total, step):
    return [(o, min(step, total - o)) for o in range(0, total, step)]


def build_mha_body(tc, ins, y_ap, b_sz=B, n_sz=N, d_sz=D, dv=DV, hloc=HLOC,
                   fill_div=NCORES):
    """Emit the per-core MHA program into TileContext tc.

    ins: dict of dram APs: xt_hi/xt_lo [b, d, n], wqt_hi/lo, wkt_hi/lo
    [hloc, d, d] (wqt pre-scaled), wv_hi/lo [hloc, d, dv], wo_hi/lo
    [hloc*dv, d].  y_ap: [b, n, d] f32 output.
    """
    nc = tc.nc
    nch_d = d_sz // P
    nch_n = n_sz // P
    half = min(FREE, n_sz)
    assert hloc * dv <= P
    fill = float(LARGE_NEG / fill_div)

    import contextlib
    ctx = contextlib.ExitStack()
    with ctx:
        p_big = ctx.enter_context(tc.tile_pool(name="big", bufs=4))
        p_sraw = ctx.enter_context(tc.tile_pool(name="sraw", bufs=2))
        p_ahi = ctx.enter_context(tc.tile_pool(name="ahi", bufs=1))
        p_m = ctx.enter_context(tc.tile_pool(name="mpool", bufs=1))
        p_xt = ctx.enter_context(tc.tile_pool(name="xt", bufs=2))
        p_wq = ctx.enter_context(tc.tile_pool(name="wq", bufs=2))
        p_v = ctx.enter_context(tc.tile_pool(name="vpool", bufs=1))
        p_wv = ctx.enter_context(tc.tile_pool(name="wv", bufs=1))
        p_wo = ctx.enter_context(tc.tile_pool(name="wo", bufs=1))
        p_cat = ctx.enter_context(tc.tile_pool(name="cat", bufs=1))
        p_misc = ctx.enter_context(tc.tile_pool(name="misc", bufs=2))
        p_misc1 = ctx.enter_context(tc.tile_pool(name="misc1", bufs=1))
        p_y = ctx.enter_context(tc.tile_pool(name="yout", bufs=2))
        ps = ctx.enter_context(tc.tile_pool(name="ps", bufs=6, space="PSUM"))
        ps_v = ctx.enter_context(tc.tile_pool(name="psv", bufs=2, space="PSUM"))

        # Wo resident (local head rows), natural layout [hv, d]
        wo_hi = p_wo.tile([hloc * dv, d_sz], BF, tag="wo_hi")
        wo_lo = p_wo.tile([hloc * dv, d_sz], BF, tag="wo_lo")
        nc.sync.dma_start(wo_hi[:], ins["wo_hi"][:])
        nc.sync.dma_start(wo_lo[:], ins["wo_lo"][:])
        fill_tile = p_wo.tile([P, FREE // 2], F32, tag="fill_tile")
        nc.gpsimd.memset(fill_tile[:], fill)

        concat_tiles = {}
        last_xt = None
        pending = []  # deferred AV/out-proj emitters, flushed after the next
        # slab's TT matmuls so the PE never waits on a softmax chain

        for hl in range(hloc):
            # ---- per-head Wv [d, dv] -> [P, nch_d, dv]
            wv_hi = p_wv.tile([P, nch_d, dv], BF, tag="wv_hi")
            wv_lo = p_wv.tile([P, nch_d, dv], BF, tag="wv_lo")
            nc.sync.dma_start(wv_hi[:], ins["wv_hi"][hl].rearrange("(c p) v -> p c v", p=P))
            nc.sync.dma_start(wv_lo[:], ins["wv_lo"][hl].rearrange("(c p) v -> p c v", p=P))

            # ---- M phase: M[d, d'] = sum_e WqT[e, d] WkT[e, d']
            m_hi = p_m.tile([P, nch_d, d_sz], BF, tag="m_hi")
            m_lo = p_m.tile([P, nch_d, d_sz], BF, tag="m_lo")
            for (dpo, dps) in _fsplits(d_sz, FREE):
                wk_hi = p_big.tile([P, nch_d, half], BF, tag="big")
                wk_lo = p_big.tile([P, nch_d, half], BF, tag="big")
                for c in range(nch_d):
                    nc.sync.dma_start(
                        wk_hi[:, c, :dps],
                        ins["wkt_hi"][hl][c * P:(c + 1) * P, dpo:dpo + dps])
                    nc.sync.dma_start(
                        wk_lo[:, c, :dps],
                        ins["wkt_lo"][hl][c * P:(c + 1) * P, dpo:dpo + dps])
                for dc in range(nch_d):
                    wq_hi = p_wq.tile([P, nch_d, P], BF, tag="wq_hi")
                    wq_lo = p_wq.tile([P, nch_d, P], BF, tag="wq_lo")
                    nc.sync.dma_start(
                        wq_hi[:],
                        ins["wqt_hi"][hl][:, dc * P:(dc + 1) * P].rearrange("(c p) f -> p c f", p=P))
                    nc.sync.dma_start(
                        wq_lo[:],
                        ins["wqt_lo"][hl][:, dc * P:(dc + 1) * P].rearrange("(c p) f -> p c f", p=P))
                    pst = ps.tile([P, FREE], F32, tag="ps")
                    for e in range(nch_d):
                        nc.tensor.matmul(pst[:, :dps], wq_hi[:, e, :], wk_hi[:, e, :dps],
                                         start=(e == 0), stop=False)
                        nc.tensor.matmul(pst[:, :dps], wq_hi[:, e, :], wk_lo[:, e, :dps],
                                         start=False, stop=False)
                        nc.tensor.matmul(pst[:, :dps], wq_lo[:, e, :], wk_hi[:, e, :dps],
                                         start=False, stop=(e == nch_d - 1))
                    nc.scalar.copy(m_hi[:, dc, dpo:dpo + dps], pst[:, :dps])
                    nc.vector.tensor_sub(m_lo[:, dc, dpo:dpo + dps], pst[:, :dps],
                                         m_hi[:, dc, dpo:dpo + dps])

            # ---- attention phase (snake order so the head boundary reuses
            # the resident X^T tile of the last batch)
            border = range(b_sz) if hl % 2 == 0 else range(b_sz - 1, -1, -1)
            for b in border:
                if last_xt is not None and last_xt[0] == b:
                    _, xt_hi, xt_lo = last_xt
                else:
                    xt_hi = p_xt.tile([P, nch_d, n_sz], BF, tag="xt_hi")
                    xt_lo = p_xt.tile([P, nch_d, n_sz], BF, tag="xt_lo")
                    nc.sync.dma_start(xt_hi[:], ins["xt_hi"][b].rearrange("(c p) n -> p c n", p=P))
                    nc.sync.dma_start(xt_lo[:], ins["xt_lo"][b].rearrange("(c p) n -> p c n", p=P))
                last_xt = (b, xt_hi, xt_lo)

                if b not in concat_tiles:
                    concat_tiles[b] = p_cat.tile([P, n_sz], BF, tag=f"cat{b}",
                                                 name=f"cat{b}")
                cat = concat_tiles[b]

                v_hi = v_lo = None
                for (nho, nhs) in _fsplits(n_sz, half):
                    # TT[d', n-half] = sum_d M[d, d'] XT[d, n]
                    tt_hi = p_big.tile([P, nch_d, half], BF, tag="big")
                    tt_lo = p_big.tile([P, nch_d, half], BF, tag="big")
                    for dp in range(nch_d):
                        pst = ps.tile([P, FREE], F32, tag="ps")
                        for dc in range(nch_d):
                            mh = m_hi[:, dc, dp * P:(dp + 1) * P]
                            ml = m_lo[:, dc, dp * P:(dp + 1) * P]
                            xh = xt_hi[:, dc, nho:nho + nhs]
                            xl = xt_lo[:, dc, nho:nho + nhs]
                            nc.tensor.matmul(pst[:, :nhs], mh, xh, start=(dc == 0), stop=False)
                            nc.tensor.matmul(pst[:, :nhs], mh, xl, start=False, stop=False)
                            nc.tensor.matmul(pst[:, :nhs], ml, xh, start=False,
                                             stop=(dc == nch_d - 1))
                        nc.scalar.copy(tt_hi[:, dp, :nhs], pst[:, :nhs])
                        nc.vector.tensor_sub(tt_lo[:, dp, :nhs], pst[:, :nhs], tt_hi[:, dp, :nhs])

                    for fn in pending:
                        fn()
                    pending.clear()

                    # scoresT[m, n-half] (+ V on the first n-half)
                    sraw = p_sraw.tile([P, nch_n, half], F32, tag="sraw")
                    runmax = p_misc1.tile([P, half], F32, tag="runmax")
                    do_v = v_hi is None
                    if do_v:
                        v_hi = p_v.tile([P, nch_n, dv], BF, tag="v_hi")
                        v_lo = p_v.tile([P, nch_n, dv], BF, tag="v_lo")
                    for mc in range(nch_n):
                        pst = ps.tile([P, FREE], F32, tag="ps")
                        if do_v:
                            pvt = ps_v.tile([P, dv], F32, tag="psv")
                        for c in range(nch_d):
                            xh = xt_hi[:, c, mc * P:(mc + 1) * P]
                            xl = xt_lo[:, c, mc * P:(mc + 1) * P]
                            nc.tensor.matmul(pst[:, :nhs], xh, tt_hi[:, c, :nhs],
                                             start=(c == 0), stop=False)
                            nc.tensor.matmul(pst[:, :nhs], xh, tt_lo[:, c, :nhs],
                                             start=False, stop=False)
                            nc.tensor.matmul(pst[:, :nhs], xl, tt_hi[:, c, :nhs],
                                             start=False, stop=(c == nch_d - 1))
                            if do_v:
                                nc.tensor.matmul(pvt[:], xh, wv_hi[:, c, :],
                                                 start=(c == 0), stop=False)
                                nc.tensor.matmul(pvt[:], xh, wv_lo[:, c, :],
                                                 start=False, stop=False)
                                nc.tensor.matmul(pvt[:], xl, wv_hi[:, c, :],
                                                 start=False, stop=(c == nch_d - 1))
                        nc.scalar.copy(sraw[:, mc, :nhs], pst[:, :nhs])
                        if mc == 0:
                            nc.vector.tensor_copy(runmax[:, :nhs], sraw[:, 0, :nhs])
                        else:
                            nc.vector.tensor_max(runmax[:, :nhs], runmax[:, :nhs],
                                                 sraw[:, mc, :nhs])
                        if do_v:
                            nc.scalar.copy(v_hi[:, mc, :], pvt[:])
                            nc.vector.tensor_sub(v_lo[:, mc, :], pvt[:], v_hi[:, mc, :])

                    # softmax over m (partition axis x chunk axis)
                    maxb = p_misc1.tile([P, half], F32, tag="maxb")
                    nc.gpsimd.partition_all_reduce(maxb[:, :nhs], runmax[:, :nhs], P,
                                                   bass_isa.ReduceOp.max)
                    a_hi = p_ahi.tile([P, nch_n, half], BF, tag="a_hi")
                    s1 = p_misc1.tile([P, half], F32, tag="s1")
                    for mc in range(nch_n):
                        nc.vector.tensor_sub(sraw[:, mc, :nhs], sraw[:, mc, :nhs],
                                             maxb[:, :nhs])
                        nc.scalar.activation(a_hi[:, mc, :nhs], sraw[:, mc, :nhs],
                                             mybir.ActivationFunctionType.Exp)
                        if mc == 0:
                            nc.vector.tensor_copy(s1[:, :nhs], a_hi[:, 0, :nhs])
                        else:
                            nc.vector.tensor_add(s1[:, :nhs], s1[:, :nhs],
                                                 a_hi[:, mc, :nhs])
                    denb = p_misc1.tile([P, half], F32, tag="denb")
                    nc.gpsimd.partition_all_reduce(denb[:, :nhs], s1[:, :nhs], P,
                                                   bass_isa.ReduceOp.add)
                    recip = p_misc1.tile([P, half], F32, tag="recip")
                    nc.vector.reciprocal(recip[:dv, :nhs], denb[:dv, :nhs])

                    # HhT[v, n-half] = sum_m V[m, v] A'[m, n] -- deferred
                    def emit_av(v_hi=v_hi, v_lo=v_lo, a_hi=a_hi, recip=recip,
                                cat=cat, hl=hl, nho=nho, nhs=nhs):
                        psav = ps.tile([P, FREE], F32, tag="ps", name="psav")
                        for mc in range(nch_n):
                            nc.tensor.matmul(psav[:dv, :nhs], v_hi[:, mc, :],
                                             a_hi[:, mc, :nhs],
                                             start=(mc == 0), stop=False)
                            nc.tensor.matmul(psav[:dv, :nhs], v_lo[:, mc, :],
                                             a_hi[:, mc, :nhs],
                                             start=False, stop=(mc == nch_n - 1))
                        nc.vector.tensor_mul(cat[hl * dv:(hl + 1) * dv, nho:nho + nhs],
                                             psav[:dv, :nhs], recip[:dv, :nhs])
                    pending.append(emit_av)

                # ---- output projection for batch b once all heads are done
                if hl == hloc - 1:
                    def emit_outproj(cat=cat, b=b):
                        for ncc in range(nch_n):
                            ct = cat[:, ncc * P:(ncc + 1) * P]
                            for (dho, dhs) in _fsplits(d_sz, FREE):
                                if dho >= ncc * P + P:
                                    # fully masked block: constant fill, no matmul
                                    for fo in range(0, dhs, FREE // 2):
                                        fs = min(FREE // 2, dhs - fo)
                                        nc.sync.dma_start(
                                            y_ap[b, ncc * P:(ncc + 1) * P,
                                                 dho + fo:dho + fo + fs],
                                            fill_tile[:, :fs])
                                    continue
                                pst = ps.tile([P, FREE], F32, tag="ps", name="psy")
                                nc.tensor.matmul(pst[:, :dhs], ct, wo_hi[:, dho:dho + dhs],
                                                 start=True, stop=False)
                                nc.tensor.matmul(pst[:, :dhs], ct, wo_lo[:, dho:dho + dhs],
                                                 start=False, stop=True)
                                yt = p_y.tile([P, FREE], F32, tag="yt", name="yt")
                                nc.scalar.copy(yt[:, :dhs], pst[:, :dhs])
                                # keep where global_row - global_col >= 0, else fill
                                nc.gpsimd.affine_select(
                                    out=yt[:, :dhs], in_=yt[:, :dhs],
                                    compare_op=mybir.AluOpType.is_ge,
                                    fill=fill, base=ncc * P - dho,
                                    pattern=[[-1, dhs]], channel_multiplier=1)
                                nc.sync.dma_start(
                                    y_ap[b, ncc * P:(ncc + 1) * P, dho:dho + dhs],
                                    yt[:, :dhs])
                    pending.append(emit_outproj)

        for fn in pending:
            fn()
        pending.clear()


def build_program(b_sz=B, n_sz=N, d_sz=D, dv=DV, hloc=HLOC, fill_div=NCORES,
                  num_devices=NCORES):
    nc = bacc.Bacc("TRN2", target_bir_lowering=False, debug=False,
                   num_devices=num_devices)
    hv = hloc * dv
    specs = {
        "xt_hi": ([b_sz, d_sz, n_sz], BF),
        "xt_lo": ([b_sz, d_sz, n_sz], BF),
        "wqt_hi": ([hloc, d_sz, d_sz], BF),
        "wqt_lo": ([hloc, d_sz, d_sz], BF),
        "wkt_hi": ([hloc, d_sz, d_sz], BF),
        "wkt_lo": ([hloc, d_sz, d_sz], BF),
        "wv_hi": ([hloc, d_sz, dv], BF),
        "wv_lo": ([hloc, d_sz, dv], BF),
        "wo_hi": ([hv, d_sz], BF),
        "wo_lo": ([hv, d_sz], BF),
    }
    ins = {k: nc.dram_tensor(k, shp, dt, kind="ExternalInput").ap()
           for k, (shp, dt) in specs.items()}
    y = nc.dram_tensor("y", [b_sz, n_sz, d_sz], F32, kind="ExternalOutput").ap()
    with tile.TileContext(nc) as tc:
        build_mha_body(tc, ins, y, b_sz=b_sz, n_sz=n_sz, d_sz=d_sz, dv=dv,
                       hloc=hloc, fill_div=fill_div)
    nc.compile()
    return nc


def _split(x):
    hi = x.astype(bf16)
    lo = (x - hi.astype(np.float32)).astype(bf16)
    return np.ascontiguousarray(hi), np.ascontiguousarray(lo)


def make_in_maps(X, W_q, W_k, W_v, W_o, ncores=NCORES, hloc=HLOC):
    scale = np.float32(1.0 / np.sqrt(X.shape[2]))
    xt = np.ascontiguousarray(X.transpose(0, 2, 1))
    xt_hi, xt_lo = _split(xt)
    in_maps = []
    for c in range(ncores):
        hs = slice(c * hloc, (c + 1) * hloc)
        wqt = np.ascontiguousarray((W_q[hs] * scale).transpose(0, 2, 1))
        wkt = np.ascontiguousarray(W_k[hs].transpose(0, 2, 1))
        wqt_hi, wqt_lo = _split(wqt)
        wkt_hi, wkt_lo = _split(wkt)
        wv_hi, wv_lo = _split(np.ascontiguousarray(W_v[hs]))
        wo_hi, wo_lo = _split(np.ascontiguousarray(
            W_o[c * hloc * W_v.shape[2]:(c + 1) * hloc * W_v.shape[2]]))
        in_maps.append({
            "xt_hi": xt_hi, "xt_lo": xt_lo,
            "wqt_hi": wqt_hi, "wqt_lo": wqt_lo,
            "wkt_hi": wkt_hi, "wkt_lo": wkt_lo,
            "wv_hi": wv_hi, "wv_lo": wv_lo,
            "wo_hi": wo_hi, "wo_lo": wo_lo,
        })
    return in_maps


_CACHE = {}


def kernel(X, W_q, W_k, W_v, W_o, _trace=False):
    from concourse.bass_utils import run_bass_kernel_spmd
    X = np.asarray(X, dtype=np.float32)
    W_q = np.asarray(W_q, dtype=np.float32)
    W_k = np.asarray(W_k, dtype=np.float32)
    W_v = np.asarray(W_v, dtype=np.float32)
    W_o = np.asarray(W_o, dtype=np.float32)

    if "nc" not in _CACHE:
        _CACHE["nc"] = build_program()
    nc = _CACHE["nc"]

    in_maps = make_in_maps(X, W_q, W_k, W_v, W_o)
    res = run_bass_kernel_spmd(nc, in_maps, list(range(NCORES)), trace=_trace)
    parts = [r["y"].astype(np.float32) for r in res.results]
    out = parts[0]
    for p in parts[1:]:
        out = out + p
    if _trace:
        _CACHE["last_result"] = res
    return out



# revision 9
# speedup vs baseline: 2.4983x; 2.4983x over previous
"""Trainium2 Bass kernel for the MultiHeadAttention problem.

Math (per head h):
  scores = (X Wq_h) (X Wk_h)^T * scale = X (scale * Wq_h Wk_h^T) X^T
so we precompute M_h = (scale*Wq_h) Wk_h^T once per head (batch independent),
then per batch compute scores directly in the transposed [m, n] orientation so
softmax's reduction axis (m) lands on partitions and A feeds the A@V matmul
without transposes:
  TT[d', n] = sum_d M[d, d'] X^T[d, n]
  scoresT[m, n] = sum_d' X^T[d', m] TT[d', n]
  A = exp(scoresT - colmax) -> HhT[v, n] = sum_m V[m, v] A[m, n] / colsum
Output projection: Y_partial = concatT^T @ Wo_local, with the post-hoc triu
mask filled with LARGE_NEG/8 on every core so the host-side shard-sum yields
exactly LARGE_NEG at masked positions.

Sharding: 16 heads / 8 cores = 2 heads per core, every core handles all 4
batches; host sums the 8 partial outputs (the only cross-core reduction).

Precision: the score path (M, TT, scoresT) and V use single-pass fp32r
matmuls — TRN2's fp32r mode runs at bf16 rate (1 cycle/row for moving dim
>= 256) with ~12-bit mantissa operand precision, giving score errors ~0.3
absolute against softmax top-2 gaps of ~400, so argmax fidelity is preserved
without the old 3-pass bf16 hi/lo splits.  V is computed transposed
(VT = Wv2^T X^T, both heads stacked, moving dim 512) then PE-transposed into
key-major layout to avoid overhead-dominated free=64 matmuls.  The A/V/AV and
output-projection paths run in 1-pass bf16 (~0.5% relative, far inside
tolerance).
"""

import os
import sys

import numpy as np
import ml_dtypes

for _p in ("/opt/trn_rl_repo",):
    if os.path.isdir(_p) and _p not in sys.path:
        sys.path.insert(0, _p)

import concourse.bass as bass
import concourse.tile as tile
from concourse import bacc, bass_isa, mybir

BF = mybir.dt.bfloat16
F32 = mybir.dt.float32
F32R = mybir.dt.float32r
bf16 = ml_dtypes.bfloat16

# Problem constants (hardcoded per contract)
B, N, D, DV, H = 4, 1024, 1024, 64, 16
NCORES = 8
HLOC = H // NCORES  # heads per core
P = 128
FREE = 512  # PSUM free-dim limit for fp32 outputs
LARGE_NEG = -1.0e9


def _fsplits(total, step):
    return [(o, min(step, total - o)) for o in range(0, total, step)]


def build_mha_body(tc, ins, y_ap, b_sz=B, n_sz=N, d_sz=D, dv=DV, hloc=HLOC,
                   fill_div=NCORES):
    """Emit the per-core MHA program into TileContext tc.

    ins: dict of dram APs (f32r unless noted): xt [b, d, n], wqt/wkt
    [hloc, d, d] (wqt pre-scaled), wv2 [d, hloc*dv], wo [hloc*dv, d] (bf16).
    y_ap: [b, n, d] f32 output.
    """
    nc = tc.nc
    nch_d = d_sz // P
    nch_n = n_sz // P
    half = min(FREE, n_sz)
    hv = hloc * dv
    assert hv <= P
    fill = float(LARGE_NEG / fill_div)

    import contextlib
    ctx = contextlib.ExitStack()
    with ctx:
        p_m = ctx.enter_context(tc.tile_pool(name="mpool", bufs=1))
        p_xt = ctx.enter_context(tc.tile_pool(name="xt", bufs=2))
        p_wk = ctx.enter_context(tc.tile_pool(name="wk", bufs=1))
        p_wq = ctx.enter_context(tc.tile_pool(name="wq", bufs=2))
        p_tt = ctx.enter_context(tc.tile_pool(name="tt", bufs=1))
        p_sraw = ctx.enter_context(tc.tile_pool(name="sraw", bufs=1))
        p_a = ctx.enter_context(tc.tile_pool(name="apool", bufs=1))
        p_v = ctx.enter_context(tc.tile_pool(name="vpool", bufs=1))
        p_wv = ctx.enter_context(tc.tile_pool(name="wv", bufs=1))
        p_wo = ctx.enter_context(tc.tile_pool(name="wo", bufs=1))
        p_cat = ctx.enter_context(tc.tile_pool(name="cat", bufs=1))
        p_misc1 = ctx.enter_context(tc.tile_pool(name="misc1", bufs=1))
        p_y = ctx.enter_context(tc.tile_pool(name="yout", bufs=1))
        ps = ctx.enter_context(tc.tile_pool(name="ps", bufs=4, space="PSUM"))
        ps_v = ctx.enter_context(tc.tile_pool(name="psv", bufs=2, space="PSUM"))

        # Wo resident (local head rows), natural layout [hv, d], bf16
        wo = p_wo.tile([hv, d_sz], BF, tag="wo")
        nc.sync.dma_start(wo[:], ins["wo"][:])
        # Wv for both local heads stacked column-wise: [P, nch_d, hv] f32r
        wv2 = p_wv.tile([P, nch_d, hv], F32R, tag="wv2")
        nc.sync.dma_start(wv2[:], ins["wv2"].rearrange("(c p) v -> p c v", p=P))
        fill_tile = p_wo.tile([P, FREE // 2], F32, tag="fill_tile")
        nc.gpsimd.memset(fill_tile[:], fill)
        # identity for PE transposes: start from ones, keep only the diagonal
        ident = p_wo.tile([P, P], F32, tag="ident")
        nc.gpsimd.memset(ident[:], 1.0)
        nc.gpsimd.affine_select(
            out=ident[:], in_=ident[:], compare_op=mybir.AluOpType.is_equal,
            fill=0.0, base=0, pattern=[[-1, P]], channel_multiplier=1)

        concat_tiles = {}
        vb_tiles = {}
        last_xt = None
        pending = []  # deferred AV/out-proj emitters, flushed after the next
        # slab's TT matmuls so the PE never waits on a softmax chain

        for hl in range(hloc):
            # ---- M phase: M[d, d'] = sum_e WqT[e, d] WkT[e, d'] (f32r)
            # Wk fully resident per head; each weight byte is DMA'd exactly once.
            m_t = p_m.tile([P, nch_d, d_sz], F32R, tag="m")
            wkf = p_wk.tile([P, nch_d, d_sz], F32R, tag="wkf")
            nc.sync.dma_start(
                wkf[:], ins["wkt"][hl].rearrange("(c p) n -> p c n", p=P))
            for dc in range(nch_d):
                wq = p_wq.tile([P, nch_d, P], F32R, tag="wq")
                nc.sync.dma_start(
                    wq[:],
                    ins["wqt"][hl][:, dc * P:(dc + 1) * P].rearrange(
                        "(c p) f -> p c f", p=P))
                for (dpo, dps) in _fsplits(d_sz, FREE):
                    pst = ps.tile([P, FREE], F32, tag="ps")
                    for e in range(nch_d):
                        nc.tensor.matmul(pst[:, :dps], wq[:, e, :],
                                         wkf[:, e, dpo:dpo + dps],
                                         start=(e == 0), stop=(e == nch_d - 1))
                    nc.scalar.copy(m_t[:, dc, dpo:dpo + dps], pst[:, :dps])

            # ---- attention phase (snake order so the head boundary reuses
            # the resident X^T tile of the last batch)
            border = range(b_sz) if hl % 2 == 0 else range(b_sz - 1, -1, -1)
            for b in border:
                if last_xt is not None and last_xt[0] == b:
                    xt = last_xt[1]
                else:
                    xt = p_xt.tile([P, nch_d, n_sz], F32R, tag="xt")
                    nc.sync.dma_start(
                        xt[:], ins["xt"][b].rearrange("(c p) n -> p c n", p=P))
                last_xt = (b, xt)

                if b not in concat_tiles:
                    concat_tiles[b] = p_cat.tile([hv, n_sz], BF, tag=f"cat{b}",
                                                 name=f"cat{b}")
                cat = concat_tiles[b]

                # ---- V for both heads, once per batch (at first head):
                # VT[v2, n] = sum_d Wv2[d, v2] X^T[d, n]  (f32r, moving dim 512)
                # then PE-transpose 128x128 blocks into key-major v_both (bf16)
                if b not in vb_tiles:
                    vb_tiles[b] = p_v.tile([P, nch_n, hv], BF, tag=f"vb{b}",
                                           name=f"vb{b}")
                    vb = vb_tiles[b]
                    for (nho, nhs) in _fsplits(n_sz, half):
                        pvt = ps_v.tile([P, FREE], F32, tag="psv")
                        for c in range(nch_d):
                            nc.tensor.matmul(pvt[:hv, :nhs], wv2[:, c, :],
                                             xt[:, c, nho:nho + nhs],
                                             start=(c == 0), stop=(c == nch_d - 1))
                        vt_sb = p_misc1.tile([P, FREE], F32, tag="s1", name="vt_sb")
                        nc.scalar.copy(vt_sb[:hv, :nhs], pvt[:hv, :nhs])
                        for j in range(nhs // P):
                            ptr = ps_v.tile([P, P], F32, tag="pstr")
                            nc.tensor.transpose(
                                ptr[:, :hv], vt_sb[:hv, j * P:(j + 1) * P],
                                ident[:hv, :hv])
                            nc.scalar.copy(vb[:, nho // P + j, :], ptr[:, :hv])
                v_both = vb_tiles[b]

                for (nho, nhs) in _fsplits(n_sz, half):
                    # TT[d', n-half] = sum_d M[d, d'] XT[d, n]  (f32r)
                    tt = p_tt.tile([P, nch_d, half], F32R, tag="tt")
                    for dp in range(nch_d):
                        pst = ps.tile([P, FREE], F32, tag="ps")
                        for dc in range(nch_d):
                            nc.tensor.matmul(
                                pst[:, :nhs], m_t[:, dc, dp * P:(dp + 1) * P],
                                xt[:, dc, nho:nho + nhs],
                                start=(dc == 0), stop=(dc == nch_d - 1))
                        nc.scalar.copy(tt[:, dp, :nhs], pst[:, :nhs])

                    for fn in pending:
                        fn()
                    pending.clear()

                    # scoresT[m, n-half]  (f32r)
                    sraw = p_sraw.tile([P, nch_n, half], F32, tag="sraw")
                    runmax = p_misc1.tile([P, half], F32, tag="runmax")
                    for mc in range(nch_n):
                        pst = ps.tile([P, FREE], F32, tag="ps")
                        for c in range(nch_d):
                            nc.tensor.matmul(
                                pst[:, :nhs], xt[:, c, mc * P:(mc + 1) * P],
                                tt[:, c, :nhs],
                                start=(c == 0), stop=(c == nch_d - 1))
                        nc.scalar.copy(sraw[:, mc, :nhs], pst[:, :nhs])
                        if mc == 0:
                            nc.vector.tensor_copy(runmax[:, :nhs], sraw[:, 0, :nhs])
                        else:
                            nc.vector.tensor_max(runmax[:, :nhs], runmax[:, :nhs],
                                                 sraw[:, mc, :nhs])

                    # softmax over m (partition axis x chunk axis)
                    maxb = p_misc1.tile([P, half], F32, tag="maxb")
                    nc.gpsimd.partition_all_reduce(maxb[:, :nhs], runmax[:, :nhs], P,
                                                   bass_isa.ReduceOp.max)
                    a_t = p_a.tile([P, nch_n, half], BF, tag="a")
                    s1 = p_misc1.tile([P, half], F32, tag="s1")
                    for mc in range(nch_n):
                        nc.vector.tensor_sub(sraw[:, mc, :nhs], sraw[:, mc, :nhs],
                                             maxb[:, :nhs])
                        nc.scalar.activation(a_t[:, mc, :nhs], sraw[:, mc, :nhs],
                                             mybir.ActivationFunctionType.Exp)
                        if mc == 0:
                            nc.vector.tensor_copy(s1[:, :nhs], a_t[:, 0, :nhs])
                        else:
                            nc.vector.tensor_add(s1[:, :nhs], s1[:, :nhs],
                                                 a_t[:, mc, :nhs])
                    denb = p_misc1.tile([P, half], F32, tag="maxb", name="denb")
                    nc.gpsimd.partition_all_reduce(denb[:, :nhs], s1[:, :nhs], P,
                                                   bass_isa.ReduceOp.add)
                    recip = p_misc1.tile([P, half], F32, tag="runmax", name="recip")
                    nc.vector.reciprocal(recip[:dv, :nhs], denb[:dv, :nhs])

                    # HhT[v, n-half] = sum_m V[m, v] A[m, n] -- deferred (bf16)
                    def emit_av(v_both=v_both, a_t=a_t, recip=recip,
                                cat=cat, hl=hl, nho=nho, nhs=nhs):
                        psav = ps.tile([P, FREE], F32, tag="ps", name="psav")
                        for mc in range(nch_n):
                            nc.tensor.matmul(
                                psav[:dv, :nhs],
                                v_both[:, mc, hl * dv:(hl + 1) * dv],
                                a_t[:, mc, :nhs],
                                start=(mc == 0), stop=(mc == nch_n - 1))
                        nc.vector.tensor_mul(cat[hl * dv:(hl + 1) * dv, nho:nho + nhs],
                                             psav[:dv, :nhs], recip[:dv, :nhs])
                    pending.append(emit_av)

                # ---- output projection for batch b once all heads are done
                if hl == hloc - 1:
                    def emit_outproj(cat=cat, b=b):
                        for ncc in range(nch_n):
                            ct = cat[:, ncc * P:(ncc + 1) * P]
                            for (dho, dhs) in _fsplits(d_sz, FREE):
                                if dho >= ncc * P + P:
                                    # fully masked block: constant fill, no matmul
                                    for fo in range(0, dhs, FREE // 2):
                                        fs = min(FREE // 2, dhs - fo)
                                        nc.sync.dma_start(
                                            y_ap[b, ncc * P:(ncc + 1) * P,
                                                 dho + fo:dho + fo + fs],
                                            fill_tile[:, :fs])
                                    continue
                                pst = ps.tile([P, FREE], F32, tag="ps", name="psy")
                                nc.tensor.matmul(pst[:, :dhs], ct, wo[:, dho:dho + dhs],
                                                 start=True, stop=True)
                                yt = p_y.tile([P, FREE], F32, tag="yt", name="yt")
                                nc.scalar.copy(yt[:, :dhs], pst[:, :dhs])
                                # keep where global_row - global_col >= 0, else fill
                                nc.gpsimd.affine_select(
                                    out=yt[:, :dhs], in_=yt[:, :dhs],
                                    compare_op=mybir.AluOpType.is_ge,
                                    fill=fill, base=ncc * P - dho,
                                    pattern=[[-1, dhs]], channel_multiplier=1)
                                nc.sync.dma_start(
                                    y_ap[b, ncc * P:(ncc + 1) * P, dho:dho + dhs],
                                    yt[:, :dhs])
                    pending.append(emit_outproj)

        for fn in pending:
            fn()
        pending.clear()


def build_program(b_sz=B, n_sz=N, d_sz=D, dv=DV, hloc=HLOC, fill_div=NCORES,
                  num_devices=NCORES):
    nc = bacc.Bacc("TRN2", target_bir_lowering=False, debug=False,
                   num_devices=num_devices)
    hv = hloc * dv
    specs = {
        "xt": ([b_sz, d_sz, n_sz], F32R),
        "wqt": ([hloc, d_sz, d_sz], F32R),
        "wkt": ([hloc, d_sz, d_sz], F32R),
        "wv2": ([d_sz, hv], F32R),
        "wo": ([hv, d_sz], BF),
    }
    ins = {k: nc.dram_tensor(k, shp, dt, kind="ExternalInput").ap()
           for k, (shp, dt) in specs.items()}
    y = nc.dram_tensor("y", [b_sz, n_sz, d_sz], F32, kind="ExternalOutput").ap()
    with tile.TileContext(nc) as tc:
        build_mha_body(tc, ins, y, b_sz=b_sz, n_sz=n_sz, d_sz=d_sz, dv=dv,
                       hloc=hloc, fill_div=fill_div)
    nc.compile()
    return nc


def make_in_maps(X, W_q, W_k, W_v, W_o, ncores=NCORES, hloc=HLOC):
    scale = np.float32(1.0 / np.sqrt(X.shape[2]))
    xt = np.ascontiguousarray(X.transpose(0, 2, 1))
    dvv = W_v.shape[2]
    in_maps = []
    for c in range(ncores):
        hs = slice(c * hloc, (c + 1) * hloc)
        wqt = np.ascontiguousarray((W_q[hs] * scale).transpose(0, 2, 1))
        wkt = np.ascontiguousarray(W_k[hs].transpose(0, 2, 1))
        wv2 = np.ascontiguousarray(
            np.concatenate([W_v[c * hloc + i] for i in range(hloc)], axis=1))
        wo = np.ascontiguousarray(
            W_o[c * hloc * dvv:(c + 1) * hloc * dvv]).astype(bf16)
        in_maps.append({
            "xt": xt, "wqt": wqt, "wkt": wkt, "wv2": wv2, "wo": wo,
        })
    return in_maps


_CACHE = {}


def kernel(X, W_q, W_k, W_v, W_o, _trace=False):
    from concourse.bass_utils import run_bass_kernel_spmd
    X = np.asarray(X, dtype=np.float32)
    W_q = np.asarray(W_q, dtype=np.float32)
    W_k = np.asarray(W_k, dtype=np.float32)
    W_v = np.asarray(W_v, dtype=np.float32)
    W_o = np.asarray(W_o, dtype=np.float32)

    if "nc" not in _CACHE:
        _CACHE["nc"] = build_program()
    nc = _CACHE["nc"]

    in_maps = make_in_maps(X, W_q, W_k, W_v, W_o)
    res = run_bass_kernel_spmd(nc, in_maps, list(range(NCORES)), trace=_trace)
    parts = [r["y"].astype(np.float32) for r in res.results]
    out = parts[0]
    for p in parts[1:]:
        out = out + p
    if _trace:
        _CACHE["last_result"] = res
    return out
